# revision 15
# baseline (speedup 1.0000x reference)
# Pairwise Euclidean distance kernel for Trainium2 (Bass/Tile).
#
# Input : coordinates_batch [8, 2048, 3] f32
# Output: [8, 2048, 2048] f32, out[b,i,j] = ||c[b,i] - c[b,j]||
#
# Sharding: data-parallel over batch -- one batch element per NeuronCore.
#
# The distance matrix is symmetric, so each core computes only the
# block-upper trapezoid (strip s = rows [128s, 128s+128) x cols
# [128s, 2048)) and the host mirrors the block-lower triangle during the
# gather.  Results are written as fp16 (tolerance is 2e-2 relative; fp16
# quantization is ~5e-4), halving HBM write bytes again: 4.46 MB/core
# instead of 16.78 MB.
#
# Per-core algorithm: d^2(i,j) = n2_i + n2_j - 2 ci.cj, computed by a
# single K=13 fp16 matmul per output tile: coordinates are 2-way
# fp16-split (c = ch + cl exact to 2^-22) and the product keeps groups
# (ch,mh),(cl,mh),(ch,ml) with m = -2c (9 rows), plus 2 rows (1 x n2_j
# splits) and 2 rows (n2_i splits x 1) so the ENTIRE d^2 -- including
# the per-row n2_i -- lands in PSUM.  fp16 x fp16 products are exact in
# the f32 accumulator; residual ~1e-4 absolute.
#
# With no per-partition bias left, ACT does out = sqrt(psum + eps) in
# one instruction spanning 4 PSUM banks (4 matmuls' worth), where
# eps = 2^-13 guarantees a non-negative sqrt argument (the host zeroes
# the diagonal exactly).  K=13 < 32 lets 4 matmuls run CONCURRENTLY in
# the PE array via tile_position row-packing (weights + rhs replicated
# at partition offsets 0/32/64/96), so PE streams ~4 cols/cycle.
#
# Engine budget per core (measured-calibrated): ACT sqrt 17.4 us
# (critical), output DMA 4.46 MB at ~300+ GB/s across both HWDGE queues
# ~14 us, PE ~7 us, DVE idle.

import numpy as np

B, N, D = 8, 2048, 3
K = 13           # stacked fp16-split contraction dim
P = 128          # output row tile (partition dim)
FT = 512         # PSUM bank width in f32
NS = N // P      # 16 strips
EPS = 2.0 ** -13

_cached_nc = None


# Fixed strip -> PE row-group color.  Chosen so (a) per-color aligned-chunk
# counts are 7/7/7/7 (aligned chunks tile into 7 rainbow quadruples) and
# (b) each partial quad {1,5,9,13}/{2,6,10,14}/{3,7,11,15} is rainbow.
# This lets the weights tensor hold only each color's own 4 strips
# (W = [128, 4*128] -- no replication).
COLOR = {0: 0, 9: 0, 10: 0, 11: 0,
         4: 1, 5: 1, 6: 1, 15: 1,
         2: 2, 7: 2, 8: 2, 13: 2,
         1: 3, 3: 3, 12: 3, 14: 3}
WSTRIPS = [[0, 9, 10, 11], [4, 5, 6, 15], [2, 7, 8, 13], [1, 3, 12, 14]]
WCOL = {s: j for t in range(4) for j, s in enumerate(WSTRIPS[t])}


def _schedule():
    """Trapezoid strip s covers cols [128s, 2048). Chunks are 512-aligned
    with a partial head chunk; groups of 4 equal-width rainbow chunks map
    to the 4 PE row-group colors, 4 PSUM banks, and one ACT op.
    Items are (s, c0); slot order in the group = PSUM bank order."""
    by_color = {t: [] for t in range(4)}
    partial = {384: [], 256: [], 128: []}
    for s in range(NS):
        start = P * s
        a0 = -(-start // FT) * FT
        if a0 > start:
            partial[a0 - start].append((s, start))
        for c0 in range(a0, N, FT):
            by_color[COLOR[s]].append((s, c0))
    for t in range(4):
        by_color[t].sort(key=lambda x: (x[1], x[0]))
    # Delay strip 0's (0,0) chunk so group 0 only needs R chunk 1.
    by_color[0][0], by_color[0][1] = by_color[0][1], by_color[0][0]
    groups = [(FT, [by_color[t][k] for t in range(4)]) for k in range(7)]
    for w in (384, 256, 128):
        groups.append((w, partial[w]))
    return groups


SCHED = _schedule()
TOTC = sum(w * len(items) for w, items in SCHED)  # 17408 packed cols


def _build_nc():
    global _cached_nc
    if _cached_nc is not None:
        return _cached_nc

    import concourse.bacc as bacc
    import concourse.mybir as mybir
    import concourse.tile as tile

    nc = bacc.Bacc("TRN2", target_bir_lowering=False, debug=False)
    f16 = mybir.dt.float16
    f32 = mybir.dt.float32

    NCH = N // FT  # 4 input chunks, chunk-major so each load is a dense block
    Wd = nc.dram_tensor("wts", [P, FT], f16, kind="ExternalInput")
    Rd = nc.dram_tensor("rhs", [NCH, P, FT], f16, kind="ExternalInput")
    out = nc.dram_tensor("out", [P, TOTC], f16, kind="ExternalOutput")

    with tile.TileContext(nc) as tc:
        with (
            tc.tile_pool(name="singles", bufs=1) as singles,
            tc.tile_pool(name="blocks", bufs=4) as blocks,
            tc.tile_pool(name="psum", bufs=2, space="PSUM") as psum,
        ):
            W = singles.tile([P, FT], f16)
            R = singles.tile([P, N], f16)

            # Warm up the ACT sqrt table (1.3 us ACT_TABLE_LOAD) during boot.
            scratch = singles.tile([1, 2], f32)
            nc.gpsimd.memset(scratch, 1.0)
            nc.scalar.sqrt(scratch, scratch)

            # Per-partition eps bias for sqrt(psum + eps).
            epsb = singles.tile([P, 1], f32)
            nc.gpsimd.memset(epsb, EPS)

            # Group 0 needs only R chunk 1 + W col block 0.  Load R1 once
            # un-replicated (32 KB) and fan out to the other partition
            # blocks with small on-chip copies; W's 32 KB head goes first.
            # Later chunks ride as dense 128 KB replicated loads.
            nc.sync.dma_start(out=R[0:32, FT : 2 * FT], in_=Rd.ap()[1, 0:32])
            for t in (1, 2, 3):
                nc.sync.dma_start(
                    out=R[32 * t : 32 * t + 32, FT : 2 * FT],
                    in_=R[0:32, FT : 2 * FT],
                )
            nc.sync.dma_start(out=R[:, 0:FT], in_=Rd.ap()[0])
            nc.scalar.dma_start(out=W[:, 0:P], in_=Wd.ap()[:, 0:P])
            nc.scalar.dma_start(out=W[:, P:FT], in_=Wd.ap()[:, P:FT])
            for c in (2, 3):
                nc.gpsimd.dma_start(out=R[:, FT * c : FT * (c + 1)], in_=Rd.ap()[c])

            off = 0
            for gi, (w, items) in enumerate(SCHED):
                ps = psum.tile([P, 4, FT], f32, tag="ps")
                for slot, (s, c0) in enumerate(items):
                    t = COLOR[s]
                    nc.tensor.matmul(
                        ps[:, slot, 0:w],
                        lhsT=W[32 * t : 32 * t + K, P * WCOL[s] : P * (WCOL[s] + 1)],
                        rhs=R[32 * t : 32 * t + K, c0 : c0 + w],
                        start=True,
                        stop=True,
                        tile_position=(32 * t, 0),
                    )
                blk = blocks.tile([P, 4 * FT], f16, tag="blk")
                nc.scalar.activation(
                    blk[:, 0 : 4 * w], ps[:, :, 0:w],
                    mybir.ActivationFunctionType.Sqrt, bias=epsb,
                )
                eng = nc.sync if gi % 2 == 0 else nc.scalar
                eng.dma_start(out=out.ap()[:, off : off + 4 * w], in_=blk[:, 0 : 4 * w])
                off += 4 * w

    nc.compile()
    _cached_nc = nc
    return nc


def _augment(x: np.ndarray):
    """x: [B, N, 3] f32 -> (W [B,128,N] fp16, R [B,128,N] fp16).

    Rows 0..12 of each 32-partition group hold the K=13 stacked operands
    (replicated at partition offsets 0/32/64/96 for PE row-packing):
      lhsT = [ch(3), cl(3), ch(3), one, one, nh, nl]
      rhs  = [mh(3), mh(3), ml(3), nh, nl, one, one]
    """
    f16 = np.float16

    def split2(a):
        h = a.astype(f16).astype(np.float32)
        l = (a - h).astype(f16).astype(np.float32)
        return h, l

    nb = x.shape[0]
    c = np.transpose(x, (0, 2, 1)).astype(np.float32)                 # [B,3,N]
    m = (-2.0 * x.astype(np.float64)).astype(np.float32).transpose(0, 2, 1)
    n2 = np.sum(x.astype(np.float64) ** 2, axis=2).astype(np.float32)[:, None, :]

    ch, cl = split2(c)
    mh, ml = split2(m)
    nh, nl = split2(n2)
    one = np.ones((nb, 1, x.shape[1]), np.float32)

    lhsT = np.concatenate([ch, cl, ch, one, one, nh, nl], 1)          # [B,13,N]
    rhs = np.concatenate([mh, mh, ml, nh, nl, one, one], 1)           # [B,13,N]

    lhsT = lhsT.astype(f16)
    # W [B, 128, 512]: partition block t holds only color t's 4 strips.
    W = np.zeros((nb, P, FT), f16)
    for t in range(4):
        for j, s in enumerate(WSTRIPS[t]):
            W[:, 32 * t : 32 * t + K, P * j : P * (j + 1)] = lhsT[:, :, P * s : P * (s + 1)]
    # R replicated at the 4 partition offsets, chunk-major [B, 4, 128, 512].
    R = np.zeros((nb, P, x.shape[1]), f16)
    for t in range(4):
        R[:, 32 * t : 32 * t + K] = rhs.astype(f16)
    R = R.reshape(nb, P, 4, FT).transpose(0, 2, 1, 3)
    return np.ascontiguousarray(W), np.ascontiguousarray(R)


def _unpack(packed: np.ndarray) -> np.ndarray:
    """packed [128, 17408] fp16 -> full [2048, 2048] f32 (mirror + diag)."""
    full = np.empty((N, N), np.float32)
    off = 0
    for w, items in SCHED:
        for t, (s, c0) in enumerate(items):
            full[P * s : P * (s + 1), c0 : c0 + w] = packed[
                :, off + t * w : off + (t + 1) * w
            ].astype(np.float32)
        off += 4 * w
    for s in range(1, NS):
        full[P * s : P * (s + 1), 0 : P * s] = full[0 : P * s, P * s : P * (s + 1)].T
    full.reshape(-1)[:: N + 1] = 0.0
    return full


def run(coordinates_batch: np.ndarray, trace: bool = False):
    """Run on 8 NeuronCores; returns (output [8,2048,2048] f32, BassKernelResults)."""
    from concourse.bass_utils import run_bass_kernel_spmd

    nc = _build_nc()
    x = np.ascontiguousarray(np.asarray(coordinates_batch), dtype=np.float32)
    assert x.shape == (B, N, D), x.shape
    W, R = _augment(x)
    in_maps = [{"wts": W[b], "rhs": R[b]} for b in range(B)]
    res = run_bass_kernel_spmd(nc, in_maps, core_ids=list(range(B)), trace=trace)
    out = np.stack([_unpack(r["out"]) for r in res.results], axis=0)
    return out, res


def kernel(coordinates_batch: np.ndarray) -> np.ndarray:
    out, _ = run(coordinates_batch, trace=False)
    return out


# revision 16
# speedup vs baseline: 1.1174x; 1.1174x over previous
# Pairwise Euclidean distance kernel for Trainium2 (Bass/Tile).
#
# Input : coordinates_batch [8, 2048, 3] f32
# Output: [8, 2048, 2048] f32, out[b,i,j] = ||c[b,i] - c[b,j]||
#
# Sharding: data-parallel over batch -- one batch element per NeuronCore.
#
# The distance matrix is symmetric, so each core computes only the
# block-upper trapezoid (strip s = rows [128s, 128s+128) x cols
# [128s, 2048)) and the host mirrors the block-lower triangle during the
# gather.  Results are written as fp16 (tolerance is 2e-2 relative; fp16
# quantization is ~5e-4), halving HBM write bytes again: 4.46 MB/core
# instead of 16.78 MB.
#
# Per-core algorithm: d^2(i,j) = n2_i + n2_j - 2 ci.cj, computed by a
# single K=13 fp16 matmul per output tile: coordinates are 2-way
# fp16-split (c = ch + cl exact to 2^-22) and the product keeps groups
# (ch,mh),(cl,mh),(ch,ml) with m = -2c (9 rows), plus 2 rows (1 x n2_j
# splits) and 2 rows (n2_i splits x 1) so the ENTIRE d^2 -- including
# the per-row n2_i -- lands in PSUM.  fp16 x fp16 products are exact in
# the f32 accumulator; residual ~1e-4 absolute.
#
# With no per-partition bias left, ACT does out = sqrt(psum + eps) in
# one instruction spanning 4 PSUM banks (4 matmuls' worth), where
# eps = 2^-13 guarantees a non-negative sqrt argument (the host zeroes
# the diagonal exactly).  K=13 < 32 lets 4 matmuls run CONCURRENTLY in
# the PE array via tile_position row-packing (weights + rhs replicated
# at partition offsets 0/32/64/96), so PE streams ~4 cols/cycle.
#
# Engine budget per core (measured-calibrated): ACT sqrt 17.4 us
# (critical), output DMA 4.46 MB at ~300+ GB/s across both HWDGE queues
# ~14 us, PE ~7 us, DVE idle.

import numpy as np

B, N, D = 8, 2048, 3
K = 13           # stacked fp16-split contraction dim
P = 128          # output row tile (partition dim)
FT = 512         # PSUM bank width in f32
NS = N // P      # 16 strips
EPS = 2.0 ** -13

_cached_nc = None


# Fixed strip -> PE row-group color.  Chosen so (a) per-color aligned-chunk
# counts are 7/7/7/7 (aligned chunks tile into 7 rainbow quadruples) and
# (b) each partial quad {1,5,9,13}/{2,6,10,14}/{3,7,11,15} is rainbow.
# This lets the weights tensor hold only each color's own 4 strips
# (W = [128, 4*128] -- no replication).
COLOR = {0: 0, 9: 0, 10: 0, 11: 0,
         4: 1, 5: 1, 6: 1, 15: 1,
         2: 2, 7: 2, 8: 2, 13: 2,
         1: 3, 3: 3, 12: 3, 14: 3}
WSTRIPS = [[0, 9, 10, 11], [4, 5, 6, 15], [2, 7, 8, 13], [1, 3, 12, 14]]
WCOL = {s: j for t in range(4) for j, s in enumerate(WSTRIPS[t])}


def _schedule():
    """Trapezoid strip s covers cols [128s, 2048). Chunks are 512-aligned
    with a partial head chunk; groups of 4 equal-width rainbow chunks map
    to the 4 PE row-group colors, 4 PSUM banks, and one ACT op.
    Items are (s, c0); slot order in the group = PSUM bank order."""
    by_color = {t: [] for t in range(4)}
    partial = {384: [], 256: [], 128: []}
    for s in range(NS):
        start = P * s
        a0 = -(-start // FT) * FT
        if a0 > start:
            partial[a0 - start].append((s, start))
        for c0 in range(a0, N, FT):
            by_color[COLOR[s]].append((s, c0))
    for t in range(4):
        by_color[t].sort(key=lambda x: (x[1], x[0]))
    # Delay strip 0's (0,0) chunk so group 0 only needs R chunk 1.
    by_color[0][0], by_color[0][1] = by_color[0][1], by_color[0][0]
    groups = [(FT, [by_color[t][k] for t in range(4)]) for k in range(7)]
    for w in (384, 256, 128):
        groups.append((w, partial[w]))
    return groups


SCHED = _schedule()
TOTC = sum(w * len(items) for w, items in SCHED)  # 17408 packed cols


def _build_nc():
    global _cached_nc
    if _cached_nc is not None:
        return _cached_nc

    import concourse.bacc as bacc
    import concourse.mybir as mybir
    import concourse.tile as tile

    nc = bacc.Bacc("TRN2", target_bir_lowering=False, debug=False)
    f16 = mybir.dt.float16
    f32 = mybir.dt.float32

    NCH = N // FT  # 4 input chunks, chunk-major so each load is a dense block
    Wd = nc.dram_tensor("wts", [P, FT], f16, kind="ExternalInput")
    Rd = nc.dram_tensor("rhs", [NCH, P, FT], f16, kind="ExternalInput")
    out = nc.dram_tensor("out", [P, TOTC], f16, kind="ExternalOutput")

    with tile.TileContext(nc) as tc:
        with (
            tc.tile_pool(name="singles", bufs=1) as singles,
            tc.tile_pool(name="blocks", bufs=4) as blocks,
            tc.tile_pool(name="psum", bufs=2, space="PSUM") as psum,
        ):
            W = singles.tile([P, FT], f16)
            R = singles.tile([P, N], f16)

            # Warm up the ACT sqrt table (1.3 us ACT_TABLE_LOAD) during boot.
            scratch = singles.tile([1, 2], f32)
            nc.gpsimd.memset(scratch, 1.0)
            nc.scalar.sqrt(scratch, scratch)

            # Per-partition eps bias for sqrt(psum + eps).
            epsb = singles.tile([P, 1], f32)
            nc.gpsimd.memset(epsb, EPS)

            # Dense 128 KB chunk loads split across the three DMA paths in
            # first-need order: group 0 needs only R chunk 1 + W.  (A
            # chained load->sb2sb fanout and a 32 KB head split both LOSE:
            # any first DMA pays a ~3 us issue-to-sem floor here, so the
            # fewest dependency hops wins.)
            nc.sync.dma_start(out=R[:, FT : 2 * FT], in_=Rd.ap()[1])
            nc.sync.dma_start(out=R[:, 0:FT], in_=Rd.ap()[0])
            nc.scalar.dma_start(out=W, in_=Wd.ap())
            for c in (2, 3):
                nc.gpsimd.dma_start(out=R[:, FT * c : FT * (c + 1)], in_=Rd.ap()[c])

            off = 0
            for gi, (w, items) in enumerate(SCHED):
                ps = psum.tile([P, 4, FT], f32, tag="ps")
                for slot, (s, c0) in enumerate(items):
                    t = COLOR[s]
                    nc.tensor.matmul(
                        ps[:, slot, 0:w],
                        lhsT=W[32 * t : 32 * t + K, P * WCOL[s] : P * (WCOL[s] + 1)],
                        rhs=R[32 * t : 32 * t + K, c0 : c0 + w],
                        start=True,
                        stop=True,
                        tile_position=(32 * t, 0),
                    )
                blk = blocks.tile([P, 4 * FT], f16, tag="blk")
                nc.scalar.activation(
                    blk[:, 0 : 4 * w], ps[:, :, 0:w],
                    mybir.ActivationFunctionType.Sqrt, bias=epsb,
                )
                eng = nc.sync if gi % 2 == 0 else nc.scalar
                eng.dma_start(out=out.ap()[:, off : off + 4 * w], in_=blk[:, 0 : 4 * w])
                off += 4 * w

    nc.compile()
    _cached_nc = nc
    return nc


def _augment(x: np.ndarray):
    """x: [B, N, 3] f32 -> (W [B,128,N] fp16, R [B,128,N] fp16).

    Rows 0..12 of each 32-partition group hold the K=13 stacked operands
    (replicated at partition offsets 0/32/64/96 for PE row-packing):
      lhsT = [ch(3), cl(3), ch(3), one, one, nh, nl]
      rhs  = [mh(3), mh(3), ml(3), nh, nl, one, one]
    """
    f16 = np.float16

    def split2(a):
        h = a.astype(f16).astype(np.float32)
        l = (a - h).astype(f16).astype(np.float32)
        return h, l

    nb = x.shape[0]
    c = np.transpose(x, (0, 2, 1)).astype(np.float32)                 # [B,3,N]
    m = (-2.0 * x.astype(np.float64)).astype(np.float32).transpose(0, 2, 1)
    n2 = np.sum(x.astype(np.float64) ** 2, axis=2).astype(np.float32)[:, None, :]

    ch, cl = split2(c)
    mh, ml = split2(m)
    nh, nl = split2(n2)
    one = np.ones((nb, 1, x.shape[1]), np.float32)

    lhsT = np.concatenate([ch, cl, ch, one, one, nh, nl], 1)          # [B,13,N]
    rhs = np.concatenate([mh, mh, ml, nh, nl, one, one], 1)           # [B,13,N]

    lhsT = lhsT.astype(f16)
    # W [B, 128, 512]: partition block t holds only color t's 4 strips.
    W = np.zeros((nb, P, FT), f16)
    for t in range(4):
        for j, s in enumerate(WSTRIPS[t]):
            W[:, 32 * t : 32 * t + K, P * j : P * (j + 1)] = lhsT[:, :, P * s : P * (s + 1)]
    # R replicated at the 4 partition offsets, chunk-major [B, 4, 128, 512].
    R = np.zeros((nb, P, x.shape[1]), f16)
    for t in range(4):
        R[:, 32 * t : 32 * t + K] = rhs.astype(f16)
    R = R.reshape(nb, P, 4, FT).transpose(0, 2, 1, 3)
    return np.ascontiguousarray(W), np.ascontiguousarray(R)


def _unpack(packed: np.ndarray) -> np.ndarray:
    """packed [128, 17408] fp16 -> full [2048, 2048] f32 (mirror + diag)."""
    full = np.empty((N, N), np.float32)
    off = 0
    for w, items in SCHED:
        for t, (s, c0) in enumerate(items):
            full[P * s : P * (s + 1), c0 : c0 + w] = packed[
                :, off + t * w : off + (t + 1) * w
            ].astype(np.float32)
        off += 4 * w
    for s in range(1, NS):
        full[P * s : P * (s + 1), 0 : P * s] = full[0 : P * s, P * s : P * (s + 1)].T
    full.reshape(-1)[:: N + 1] = 0.0
    return full


def run(coordinates_batch: np.ndarray, trace: bool = False):
    """Run on 8 NeuronCores; returns (output [8,2048,2048] f32, BassKernelResults)."""
    from concourse.bass_utils import run_bass_kernel_spmd

    nc = _build_nc()
    x = np.ascontiguousarray(np.asarray(coordinates_batch), dtype=np.float32)
    assert x.shape == (B, N, D), x.shape
    W, R = _augment(x)
    in_maps = [{"wts": W[b], "rhs": R[b]} for b in range(B)]
    res = run_bass_kernel_spmd(nc, in_maps, core_ids=list(range(B)), trace=trace)
    out = np.stack([_unpack(r["out"]) for r in res.results], axis=0)
    return out, res


def kernel(coordinates_batch: np.ndarray) -> np.ndarray:
    out, _ = run(coordinates_batch, trace=False)
    return out
